# revision 11
# baseline (speedup 1.0000x reference)
"""CTC batch cost on 8 Trainium2 NeuronCores.

Algorithm (prob-space CTC forward/backward, s-major time-scan):
- B=256, T=512, C=100, U=32 -> S=2U+1=65 extended states, blank=99.
- Emissions gathered + normalized on host: p~[b,t,s] = (y[b,t,ext[s]]+1e-7)/(mu*mean_s),
  with per-direction mu (centers the time-drift). log r re-added on host.
- Per-example alignment: T - input_len dummy steps PREPENDED (one-hot emission at
  s=0 keeps alpha fixed), so every example's DP ends at position 511.
- 8 cores = 4 batch groups (64 examples) x 2 directions:
    fwd: alpha DP positions 0..255 (ascending states)
    bwd: gamma DP positions 511..256, time+state reversed on host so the
         device program is identical.
- Device per state-column (65 cols):
    v = (col[s-2]*m_s) + col[s-1]                   (DVE scalar_tensor_tensor)
    col[s] = scan_t(state = (v[t]+state)*p~[t])     (DVE tensor_tensor_scan)
  plus an adaptive rescale every 8 columns (reduce_max -> reciprocal -> scale
  the two boundary columns and pending init slots to peak ~1e28); the scale
  factors ship to the host, which undoes them in f64.
- Host splice: P = sum_s A255[s]*(G[s]+G[s+1]+m[s+2]G[s+2]);
  loss = -(log P + sum log r).
"""

import numpy as np

B, T, C, U = 256, 512, 100, 32
S = 2 * U + 1
BLANK = C - 1
TH = T // 2          # 256 positions per direction
NB = B // 4          # 64 examples per core
SLOT = TH + 1        # 257 slots per column (slot 0 = init)
GCOLS = S + 2        # 67 columns incl. 2 zero guard columns
RMULT_F = 1.83
RMULT_B = 1.50
BOUND_COLS = (7, 15, 23, 31, 39, 47, 55)
TARGET = 1e28

_CACHE = {}


def _build_nc():
    import concourse.bacc as bacc
    import concourse.mybir as mybir
    from concourse.tile import TileContext

    f32 = mybir.dt.float32
    mult = mybir.AluOpType.mult
    add = mybir.AluOpType.add

    nc = bacc.Bacc("TRN2", target_bir_lowering=False, debug=False)
    pemit = nc.dram_tensor("pemit", [NB, S * TH], f32, kind="ExternalInput")
    mtab = nc.dram_tensor("mtab", [NB, S], f32, kind="ExternalInput")
    inittab = nc.dram_tensor("inittab", [NB, S], f32, kind="ExternalInput")
    lasts = nc.dram_tensor("lasts", [NB, S], f32, kind="ExternalOutput")
    rhod = nc.dram_tensor("rho", [NB, len(BOUND_COLS)], f32, kind="ExternalOutput")

    NCH = 5  # columns per pemit DMA chunk -> 13 chunks
    with TileContext(nc) as tc:
        with (
            tc.tile_pool(name="persist", bufs=1) as pp,
            tc.tile_pool(name="scratch", bufs=3) as sp,
        ):
            cols = pp.tile([NB, GCOLS * SLOT], f32)
            mt = pp.tile([NB, S], f32)
            init_sb = pp.tile([NB, S], f32)
            rho_sb = pp.tile([NB, len(BOUND_COLS)], f32)
            pe = []
            for g in range(13):
                t = pp.tile([NB, NCH * TH], f32, tag=f"pe{g}")
                nc.sync.dma_start(
                    out=t[:, :], in_=pemit[:, g * NCH * TH:(g + 1) * NCH * TH]
                )
                pe.append(t)
            nc.sync.dma_start(out=mt[:, :], in_=mtab[:, :])
            nc.sync.dma_start(out=init_sb[:, :], in_=inittab[:, :])

            # zero the two guard columns (incl. their slot 0)
            nc.vector.memset(cols[:, 0:2 * SLOT], 0.0)
            # write init values into slot 0 of every real column
            cols3 = cols.rearrange("p (c t) -> p c t", t=SLOT)
            init3 = init_sb.rearrange("p (c o) -> p c o", o=1)
            nc.vector.tensor_copy(out=cols3[:, 2:2 + S, 0:1], in_=init3[:, :, :])

            for col in range(S):
                c = col + 2
                sh2 = cols[:, (c - 2) * SLOT:(c - 2) * SLOT + TH]
                sh1 = cols[:, (c - 1) * SLOT:(c - 1) * SLOT + TH]
                g, off = col // NCH, (col % NCH) * TH
                d0 = pe[g][:, off:off + TH]
                if col % 2 == 0:
                    # blank column: can_skip mask is always 0 -> v = sh1
                    vap = sh1
                else:
                    v = sp.tile([NB, TH], f32, tag="v")
                    nc.vector.scalar_tensor_tensor(
                        out=v[:, :], in0=sh2, scalar=mt[:, col:col + 1], in1=sh1,
                        op0=mult, op1=add,
                    )
                    vap = v[:, :]
                # scan: state = (v[t] + state) * p~[t]  == the CTC column update
                nc.vector.tensor_tensor_scan(
                    out=cols[:, c * SLOT + 1:c * SLOT + 1 + TH],
                    data0=vap, data1=d0,
                    initial=cols[:, c * SLOT:c * SLOT + 1],
                    op0=add, op1=mult,
                )

                if col in BOUND_COLS:
                    gi = BOUND_COLS.index(col)
                    mx = sp.tile([NB, 1], f32, tag="mx")
                    mxc = sp.tile([NB, 1], f32, tag="mxc")
                    msk = sp.tile([NB, 1], f32, tag="msk")
                    mx2 = sp.tile([NB, 1], f32, tag="mx2")
                    colap = cols[:, c * SLOT:c * SLOT + SLOT]
                    nc.vector.tensor_reduce(
                        out=mx[:, :], in_=colap, op=mybir.AluOpType.max,
                        axis=mybir.AxisListType.X,
                    )
                    nc.vector.tensor_scalar_max(mxc[:, :], mx[:, :], 1e-30)
                    nc.vector.tensor_scalar(
                        out=msk[:, :], in0=mx[:, :], scalar1=0.0, scalar2=None,
                        op0=mybir.AluOpType.is_le,
                    )
                    # mx2 = clamp(mx) + (mx<=0)*TARGET  (empty col -> ~TARGET)
                    nc.vector.scalar_tensor_tensor(
                        out=mx2[:, :], in0=msk[:, :], scalar=float(TARGET),
                        in1=mxc[:, :], op0=mult, op1=add,
                    )
                    # ship the exact inv used so the host undo is error-free
                    nc.vector.reciprocal(rho_sb[:, gi:gi + 1], mx2[:, :])
                    inv_ap = rho_sb[:, gi:gi + 1]
                    # x = (x * inv) * TARGET for both boundary columns (adjacent)
                    both = cols[:, (c - 1) * SLOT:(c + 1) * SLOT]
                    nc.vector.tensor_scalar(
                        out=both, in0=both, scalar1=inv_ap,
                        scalar2=float(TARGET), op0=mult, op1=mult,
                    )
                    # pending init slots of later columns inherit the scale
                    nc.vector.tensor_scalar(
                        out=cols3[:, c + 1:, 0:1], in0=cols3[:, c + 1:, 0:1],
                        scalar1=inv_ap, scalar2=float(TARGET),
                        op0=mult, op1=mult,
                    )

            lasts3 = lasts[:, :].rearrange("p (c o) -> p c o", o=1)
            nc.sync.dma_start(out=lasts3, in_=cols3[:, 2:2 + S, TH:TH + 1])
            nc.sync.dma_start(out=rhod[:, :], in_=rho_sb[:, :])
    nc.finalize()
    return nc


def _host_prep(y_pred, labels, input_length, label_length):
    f32 = np.float32
    yp = np.asarray(y_pred, f32)
    lab = np.asarray(labels, np.int32)
    ilen = np.asarray(input_length, np.int32).reshape(B)
    llen = np.asarray(label_length, np.int32).reshape(B)

    ext = np.full((B, S), BLANK, np.int32)
    ext[:, 1::2] = lab
    emit = np.take_along_axis(yp, ext[:, None, :], axis=2) + f32(1e-7)  # [B,T,S]
    rm = emit.mean(axis=2, dtype=np.float32).astype(f32)                # [B,T]
    pn_f = emit / (f32(RMULT_F) * rm[:, :, None])
    pn_b = emit / (f32(RMULT_B) * rm[:, :, None])

    prev2 = np.concatenate([np.full((B, 2), -1, np.int32), ext[:, :-2]], axis=1)
    m = ((ext != BLANK) & (ext != prev2)).astype(f32)                   # [B,S]

    n_dummy = (T - ilen).astype(np.int32)
    pos = np.arange(T)
    t_idx = pos[None, :] - n_dummy[:, None]
    dummy = t_idx < 0
    t_safe = np.clip(t_idx, 0, T - 1)
    bi = np.arange(B)[:, None]
    Pfull_f = pn_f[bi, t_safe, :]                                       # [B,T,S]
    onehot0 = np.zeros((S,), f32)
    onehot0[0] = 1.0
    Pfull_f[dummy] = onehot0

    Pf = np.ascontiguousarray(Pfull_f[:, :TH, :].transpose(0, 2, 1))    # [B,S,TH]
    init_f = np.zeros((B, S), f32)
    init_f[:, 0] = f32(TARGET)

    Pb = np.ascontiguousarray(
        pn_b[bi, t_safe, :][:, TH:, :][:, ::-1, :].transpose(0, 2, 1)[:, ::-1, :]
    )                                                                   # [B,S,TH] j-major
    m_b = np.zeros((B, S), f32)
    js = np.arange(2, S)
    m_b[:, js] = m[:, 66 - js]
    init_b = np.zeros((B, S), f32)
    init_b[np.arange(B), S - 1 - 2 * llen] = f32(TARGET)

    tmask = pos[None, :] < ilen[:, None]
    logr_sum = ((np.log(rm.astype(np.float64)) * tmask).sum(axis=1)
                + (ilen - TH) * np.log(RMULT_F) + TH * np.log(RMULT_B))
    return Pf, m, init_f, Pb, m_b, init_b, logr_sum


def _undo_scales(lasts, rho):
    """rho holds the exact inv each boundary applied; stored values carry
    TARGET (init) and prod (inv_g*TARGET) factors -> divide them out in f64."""
    logc = np.full((lasts.shape[0], S), -np.log(TARGET))
    lr = np.log(rho.astype(np.float64)) + np.log(TARGET)
    for g, jg in enumerate(BOUND_COLS):
        logc[:, jg - 1:] -= lr[:, g][:, None]
    return lasts.astype(np.float64) * np.exp(logc)


def kernel(y_pred, labels, input_length, label_length):
    from concourse.bass_utils import run_bass_kernel_spmd

    Pf, m_f, init_f, Pb, m_b, init_b, logr_sum = _host_prep(
        y_pred, labels, input_length, label_length
    )

    in_maps = []
    for core in range(8):
        g = core % 4
        sl = slice(g * NB, (g + 1) * NB)
        if core < 4:
            P, mm, ii = Pf[sl], m_f[sl], init_f[sl]
        else:
            P, mm, ii = Pb[sl], m_b[sl], init_b[sl]
        in_maps.append({
            "pemit": np.ascontiguousarray(P.reshape(NB, S * TH)),
            "mtab": np.ascontiguousarray(mm),
            "inittab": np.ascontiguousarray(ii),
        })

    if "nc" not in _CACHE:
        _CACHE["nc"] = _build_nc()
    res = run_bass_kernel_spmd(_CACHE["nc"], in_maps, core_ids=list(range(8)))
    outs = res.results

    lasts_f = np.concatenate(
        [_undo_scales(outs[c]["lasts"], outs[c]["rho"]) for c in range(4)], axis=0)
    lasts_bj = np.concatenate(
        [_undo_scales(outs[c]["lasts"], outs[c]["rho"]) for c in range(4, 8)], axis=0)
    G = lasts_bj[:, ::-1]                                               # by s

    z1 = np.zeros((B, 1))
    z2 = np.zeros((B, 2))
    Gp1 = np.concatenate([G[:, 1:], z1], axis=1)
    Gp2 = np.concatenate([G[:, 2:], z2], axis=1)
    msh = np.concatenate([m_f[:, 2:].astype(np.float64), z2], axis=1)
    Bt = G + Gp1 + msh * Gp2
    Ptot = (lasts_f * Bt).sum(axis=1)
    loss = -(np.log(Ptot) + logr_sum)
    return loss.astype(np.float32).reshape(B, 1)


# revision 12
# speedup vs baseline: 1.0504x; 1.0504x over previous
"""CTC batch cost on 8 Trainium2 NeuronCores.

Algorithm (prob-space CTC forward/backward, s-major time-scan):
- B=256, T=512, C=100, U=32 -> S=2U+1=65 extended states, blank=99.
- Emissions gathered + normalized on host: p~[b,t,s] = (y[b,t,ext[s]]+1e-7)/(mu*mean_s),
  with per-direction mu (centers the time-drift). log r re-added on host.
- Per-example alignment: T - input_len dummy steps PREPENDED (one-hot emission at
  s=0 keeps alpha fixed), so every example's DP ends at position 511.
- 8 cores = 4 batch groups (64 examples) x 2 directions:
    fwd: alpha DP positions 0..255 (ascending states)
    bwd: gamma DP positions 511..256, time+state reversed on host so the
         device program is identical.
- Device per state-column (65 cols):
    v = (col[s-2]*m_s) + col[s-1]                   (DVE scalar_tensor_tensor)
    col[s] = scan_t(state = (v[t]+state)*p~[t])     (DVE tensor_tensor_scan)
  plus an adaptive rescale every 13 columns (reduce_max -> reciprocal -> scale
  the two boundary columns and pending init slots to peak ~1e28); the scale
  factors ship to the host, which undoes them in f64.
- Host splice: P = sum_s A255[s]*(G[s]+G[s+1]+m[s+2]G[s+2]);
  loss = -(log P + sum log r).
"""

import numpy as np

B, T, C, U = 256, 512, 100, 32
S = 2 * U + 1
BLANK = C - 1
TH = T // 2          # 256 positions per direction
NB = B // 4          # 64 examples per core
SLOT = TH + 1        # 257 slots per column (slot 0 = init)
GCOLS = S + 2        # 67 columns incl. 2 zero guard columns
RMULT_F = 1.83
RMULT_B = 1.50
BOUND_COLS = (12, 25, 38, 51)
TARGET = 1e28

_CACHE = {}


def _build_nc():
    import concourse.bacc as bacc
    import concourse.mybir as mybir
    from concourse.tile import TileContext

    f32 = mybir.dt.float32
    mult = mybir.AluOpType.mult
    add = mybir.AluOpType.add

    nc = bacc.Bacc("TRN2", target_bir_lowering=False, debug=False)
    pemit = nc.dram_tensor("pemit", [NB, S * TH], f32, kind="ExternalInput")
    mtab = nc.dram_tensor("mtab", [NB, S], f32, kind="ExternalInput")
    inittab = nc.dram_tensor("inittab", [NB, S], f32, kind="ExternalInput")
    lasts = nc.dram_tensor("lasts", [NB, S], f32, kind="ExternalOutput")
    rhod = nc.dram_tensor("rho", [NB, len(BOUND_COLS)], f32, kind="ExternalOutput")

    NCH = 5  # columns per pemit DMA chunk -> 13 chunks
    with TileContext(nc) as tc:
        with (
            tc.tile_pool(name="persist", bufs=1) as pp,
            tc.tile_pool(name="scratch", bufs=3) as sp,
        ):
            cols = pp.tile([NB, GCOLS * SLOT], f32)
            mt = pp.tile([NB, S], f32)
            init_sb = pp.tile([NB, S], f32)
            rho_sb = pp.tile([NB, len(BOUND_COLS)], f32)
            pe = []
            for g in range(13):
                t = pp.tile([NB, NCH * TH], f32, tag=f"pe{g}")
                nc.sync.dma_start(
                    out=t[:, :], in_=pemit[:, g * NCH * TH:(g + 1) * NCH * TH]
                )
                pe.append(t)
            nc.sync.dma_start(out=mt[:, :], in_=mtab[:, :])
            nc.sync.dma_start(out=init_sb[:, :], in_=inittab[:, :])

            # zero the two guard columns (incl. their slot 0)
            nc.vector.memset(cols[:, 0:2 * SLOT], 0.0)
            # write init values into slot 0 of every real column
            cols3 = cols.rearrange("p (c t) -> p c t", t=SLOT)
            init3 = init_sb.rearrange("p (c o) -> p c o", o=1)
            nc.vector.tensor_copy(out=cols3[:, 2:2 + S, 0:1], in_=init3[:, :, :])

            for col in range(S):
                c = col + 2
                sh2 = cols[:, (c - 2) * SLOT:(c - 2) * SLOT + TH]
                sh1 = cols[:, (c - 1) * SLOT:(c - 1) * SLOT + TH]
                g, off = col // NCH, (col % NCH) * TH
                d0 = pe[g][:, off:off + TH]
                if col % 2 == 0:
                    # blank column: can_skip mask is always 0 -> v = sh1
                    vap = sh1
                else:
                    v = sp.tile([NB, TH], f32, tag="v")
                    nc.vector.scalar_tensor_tensor(
                        out=v[:, :], in0=sh2, scalar=mt[:, col:col + 1], in1=sh1,
                        op0=mult, op1=add,
                    )
                    vap = v[:, :]
                # scan: state = (v[t] + state) * p~[t]  == the CTC column update
                nc.vector.tensor_tensor_scan(
                    out=cols[:, c * SLOT + 1:c * SLOT + 1 + TH],
                    data0=vap, data1=d0,
                    initial=cols[:, c * SLOT:c * SLOT + 1],
                    op0=add, op1=mult,
                )

                if col in BOUND_COLS:
                    gi = BOUND_COLS.index(col)
                    mx = sp.tile([NB, 1], f32, tag="mx")
                    mxc = sp.tile([NB, 1], f32, tag="mxc")
                    msk = sp.tile([NB, 1], f32, tag="msk")
                    mx2 = sp.tile([NB, 1], f32, tag="mx2")
                    colap = cols[:, c * SLOT:c * SLOT + SLOT]
                    nc.vector.tensor_reduce(
                        out=mx[:, :], in_=colap, op=mybir.AluOpType.max,
                        axis=mybir.AxisListType.X,
                    )
                    nc.vector.tensor_scalar_max(mxc[:, :], mx[:, :], 1e-30)
                    nc.vector.tensor_scalar(
                        out=msk[:, :], in0=mx[:, :], scalar1=0.0, scalar2=None,
                        op0=mybir.AluOpType.is_le,
                    )
                    # mx2 = clamp(mx) + (mx<=0)*TARGET  (empty col -> ~TARGET)
                    nc.vector.scalar_tensor_tensor(
                        out=mx2[:, :], in0=msk[:, :], scalar=float(TARGET),
                        in1=mxc[:, :], op0=mult, op1=add,
                    )
                    # ship the exact inv used so the host undo is error-free
                    nc.vector.reciprocal(rho_sb[:, gi:gi + 1], mx2[:, :])
                    inv_ap = rho_sb[:, gi:gi + 1]
                    # x = (x * inv) * TARGET for both boundary columns (adjacent)
                    both = cols[:, (c - 1) * SLOT:(c + 1) * SLOT]
                    nc.vector.tensor_scalar(
                        out=both, in0=both, scalar1=inv_ap,
                        scalar2=float(TARGET), op0=mult, op1=mult,
                    )
                    # pending init slots of later columns inherit the scale
                    nc.vector.tensor_scalar(
                        out=cols3[:, c + 1:, 0:1], in0=cols3[:, c + 1:, 0:1],
                        scalar1=inv_ap, scalar2=float(TARGET),
                        op0=mult, op1=mult,
                    )

            lasts3 = lasts[:, :].rearrange("p (c o) -> p c o", o=1)
            nc.sync.dma_start(out=lasts3, in_=cols3[:, 2:2 + S, TH:TH + 1])
            nc.sync.dma_start(out=rhod[:, :], in_=rho_sb[:, :])
    nc.finalize()
    return nc


def _host_prep(y_pred, labels, input_length, label_length):
    f32 = np.float32
    yp = np.asarray(y_pred, f32)
    lab = np.asarray(labels, np.int32)
    ilen = np.asarray(input_length, np.int32).reshape(B)
    llen = np.asarray(label_length, np.int32).reshape(B)

    ext = np.full((B, S), BLANK, np.int32)
    ext[:, 1::2] = lab
    emit = np.take_along_axis(yp, ext[:, None, :], axis=2) + f32(1e-7)  # [B,T,S]
    rm = emit.mean(axis=2, dtype=np.float32).astype(f32)                # [B,T]
    pn_f = emit / (f32(RMULT_F) * rm[:, :, None])
    pn_b = emit / (f32(RMULT_B) * rm[:, :, None])

    prev2 = np.concatenate([np.full((B, 2), -1, np.int32), ext[:, :-2]], axis=1)
    m = ((ext != BLANK) & (ext != prev2)).astype(f32)                   # [B,S]

    n_dummy = (T - ilen).astype(np.int32)
    pos = np.arange(T)
    t_idx = pos[None, :] - n_dummy[:, None]
    dummy = t_idx < 0
    t_safe = np.clip(t_idx, 0, T - 1)
    bi = np.arange(B)[:, None]
    Pfull_f = pn_f[bi, t_safe, :]                                       # [B,T,S]
    onehot0 = np.zeros((S,), f32)
    onehot0[0] = 1.0
    Pfull_f[dummy] = onehot0

    Pf = np.ascontiguousarray(Pfull_f[:, :TH, :].transpose(0, 2, 1))    # [B,S,TH]
    init_f = np.zeros((B, S), f32)
    init_f[:, 0] = f32(TARGET)

    Pb = np.ascontiguousarray(
        pn_b[bi, t_safe, :][:, TH:, :][:, ::-1, :].transpose(0, 2, 1)[:, ::-1, :]
    )                                                                   # [B,S,TH] j-major
    m_b = np.zeros((B, S), f32)
    js = np.arange(2, S)
    m_b[:, js] = m[:, 66 - js]
    init_b = np.zeros((B, S), f32)
    init_b[np.arange(B), S - 1 - 2 * llen] = f32(TARGET)

    tmask = pos[None, :] < ilen[:, None]
    logr_sum = ((np.log(rm.astype(np.float64)) * tmask).sum(axis=1)
                + (ilen - TH) * np.log(RMULT_F) + TH * np.log(RMULT_B))
    return Pf, m, init_f, Pb, m_b, init_b, logr_sum


def _undo_scales(lasts, rho):
    """rho holds the exact inv each boundary applied; stored values carry
    TARGET (init) and prod (inv_g*TARGET) factors -> divide them out in f64."""
    logc = np.full((lasts.shape[0], S), -np.log(TARGET))
    lr = np.log(rho.astype(np.float64)) + np.log(TARGET)
    for g, jg in enumerate(BOUND_COLS):
        logc[:, jg - 1:] -= lr[:, g][:, None]
    return lasts.astype(np.float64) * np.exp(logc)


def kernel(y_pred, labels, input_length, label_length):
    from concourse.bass_utils import run_bass_kernel_spmd

    Pf, m_f, init_f, Pb, m_b, init_b, logr_sum = _host_prep(
        y_pred, labels, input_length, label_length
    )

    in_maps = []
    for core in range(8):
        g = core % 4
        sl = slice(g * NB, (g + 1) * NB)
        if core < 4:
            P, mm, ii = Pf[sl], m_f[sl], init_f[sl]
        else:
            P, mm, ii = Pb[sl], m_b[sl], init_b[sl]
        in_maps.append({
            "pemit": np.ascontiguousarray(P.reshape(NB, S * TH)),
            "mtab": np.ascontiguousarray(mm),
            "inittab": np.ascontiguousarray(ii),
        })

    if "nc" not in _CACHE:
        _CACHE["nc"] = _build_nc()
    res = run_bass_kernel_spmd(_CACHE["nc"], in_maps, core_ids=list(range(8)))
    outs = res.results

    lasts_f = np.concatenate(
        [_undo_scales(outs[c]["lasts"], outs[c]["rho"]) for c in range(4)], axis=0)
    lasts_bj = np.concatenate(
        [_undo_scales(outs[c]["lasts"], outs[c]["rho"]) for c in range(4, 8)], axis=0)
    G = lasts_bj[:, ::-1]                                               # by s

    z1 = np.zeros((B, 1))
    z2 = np.zeros((B, 2))
    Gp1 = np.concatenate([G[:, 1:], z1], axis=1)
    Gp2 = np.concatenate([G[:, 2:], z2], axis=1)
    msh = np.concatenate([m_f[:, 2:].astype(np.float64), z2], axis=1)
    Bt = G + Gp1 + msh * Gp2
    Ptot = (lasts_f * Bt).sum(axis=1)
    loss = -(np.log(Ptot) + logr_sum)
    return loss.astype(np.float32).reshape(B, 1)
